# revision 1
# baseline (speedup 1.0000x reference)
"""BoxRenderLoss Trainium2 kernel.

loss = mean over (box, fragment) pairs of masked min-squared-distance between
each box's 10x10 fragment grid and the other box's 100-point sampled boundary,
both directions, / (2*B*FP).

Algorithm: the min over the 100 boundary points decomposes into the 4 box
edges; each edge's 25-point uniform grid min has the closed form
k* = clamp(round(u/s), 0, 24), val = u - s*k*.  Per (row, fragment) item:
  dmin = min( min(ux^2, vx^2) + valy^2,  min(uy^2, vy^2) + valx^2 )
  mask = min(ux, vx, uy, vy) < 0         (fragment outside other box)
  contribution = dmin * mask

Device layout: partitions = 100 fragment points, free dim = virtual rows
(4096 boxes x 2 directions, data-parallel over 8 cores -> 1024 columns/core,
2 chunks of 512, x|y packed side by side -> FD 1024).  The affine maps
U = gx*w + dx, V = -gx*w + dvx, T = gx*(w*rix) + dx*rix and broadcast SB = s
are K<=2 outer-product matmuls on the TensorEngine.  All matmul operands live
in ONE SBUF tile (rows 0-1 / 32-33 / 64-65 for the base-partition-0/32/64
groups) and each PSUM tensor is consumed by exactly one engine -- PE Matmult
instructions only support a single semaphore wait.  Final per-partition row
sums come free via scalar_tensor_tensor's accum_out; host sums 100x2x8
partials and divides.
"""

import os
import numpy as np

# Exact float32 bit patterns of jnp.linspace(0.0, 1.0, 10) (fragment grid).
_LIN10 = np.array(
    [0, 1038323257, 1046711865, 1051372203, 1055100473,
     1057896676, 1059760811, 1061624946, 1063489081, 1065353216],
    dtype=np.uint32,
).view(np.float32)

_B = 4096
_FP = 100
_N_CORES = 8
_BOX_PER_CORE = _B // _N_CORES          # 512
_COLS = 2 * _BOX_PER_CORE               # 1024 virtual rows per core
_CHUNK = 512
_N_CHUNKS = _COLS // _CHUNK             # 2
_MAGIC = 8388608.0                      # 2^23 round-to-nearest trick

# mm-input tile column layout: [lhsT 0:128 | block0 | block1 | block2]
_LW = 128
_MMW = _LW + 3 * _COLS                  # 3200

LAST_RESULTS = None  # BassKernelResults of the most recent run (for test.py)

_compiled = {}


def _build_nc():
    import concourse.bass as bass
    import concourse.bacc as bacc
    import concourse.tile as tile
    from concourse import mybir

    f32 = mybir.dt.float32
    bf16 = mybir.dt.bfloat16
    Op = mybir.AluOpType
    Act = mybir.ActivationFunctionType

    nc = bacc.Bacc("TRN2", target_bir_lowering=False, debug=False,
                   num_devices=_N_CORES)
    f32r = mybir.dt.float32r
    mmin_d = nc.dram_tensor("mmin", [6, _MMW], f32r,
                            kind="ExternalInput").ap()
    out_d = nc.dram_tensor("out", [_FP, _N_CHUNKS], f32,
                           kind="ExternalOutput").ap()

    def blk(b, c):  # rhs slice columns for block b, chunk c
        start = _LW + b * _COLS + c * _CHUNK
        return slice(start, start + _CHUNK)

    from concourse.tile import add_dep_helper

    with tile.TileContext(nc) as tc:
        with (
            tc.tile_pool(name="const", bufs=1) as const,
            tc.tile_pool(name="sb", bufs=4) as sb,
            tc.tile_pool(name="ps", bufs=1, space="PSUM") as ps,
        ):
            mt = const.tile([66, _MMW], f32r)
            # One DMA per base-partition row group so every matmul carries at
            # most one new DMA wait.
            nc.sync.dma_start(mt[0:2, :], mmin_d[0:2, :])
            nc.sync.dma_start(mt[32:34, :], mmin_d[2:4, :])
            nc.sync.dma_start(mt[64:66, :], mmin_d[4:6, :])
            part = const.tile([_FP, _N_CHUNKS], f32)

            # float32r (same bits as f32): makes tile_legalize split each
            # Matmult into LdWeights + Matmult so semaphore waits spread
            # across two PE instructions (Matmult's LW slot fits only one).
            mtr = mt
            gx = mtr[0:2, :_FP]     # [gx; ones]
            gy = mtr[32:34, :_FP]   # [gy; ones]
            one = mtr[64:65, :_FP]  # [ones]

            for c in range(_N_CHUNKS):
                W = 2 * _CHUNK
                U = ps.tile([_FP, W], f32, tag="U")
                V = ps.tile([_FP, W], f32, tag="V")
                T = ps.tile([_FP, W], f32, tag="T")
                SB = ps.tile([_FP, W], f32, tag="SB")
                xh = slice(0, _CHUNK)
                yh = slice(_CHUNK, W)

                nc.tensor.matmul(U[:, xh], gx, mtr[0:2, blk(0, c)])
                nc.tensor.matmul(U[:, yh], gy, mtr[32:34, blk(0, c)])
                nc.tensor.matmul(V[:, xh], gx, mtr[0:2, blk(1, c)])
                nc.tensor.matmul(V[:, yh], gy, mtr[32:34, blk(1, c)])
                nc.tensor.matmul(T[:, xh], gx, mtr[0:2, blk(2, c)])
                nc.tensor.matmul(T[:, yh], gy, mtr[32:34, blk(2, c)])
                sxs = slice(_LW + c * _CHUNK, _LW + c * _CHUNK + _CHUNK)
                sys_ = slice(_LW + _COLS + c * _CHUNK,
                             _LW + _COLS + c * _CHUNK + _CHUNK)
                nc.tensor.matmul(SB[:, xh], one, mtr[64:65, sxs])
                nc.tensor.matmul(SB[:, yh], one, mtr[64:65, sys_])

                usq = sb.tile([_FP, W], bf16, tag="usq")
                nc.scalar.activation(usq[:], U[:], Act.Square)
                vs = sb.tile([_FP, W], f32, tag="vs")
                nc.scalar.activation(vs[:], V[:], Act.Copy)
                vsq = sb.tile([_FP, W], bf16, tag="vsq")
                nc.scalar.activation(vsq[:], V[:], Act.Square)

                r1 = sb.tile([_FP, W], bf16, tag="r1")
                nc.scalar.activation(r1[:], T[:], Act.Relu)
                kc1 = sb.tile([_FP, W], f32, tag="kc1")
                nc.vector.tensor_scalar(kc1[:], r1[:], _MAGIC,
                                        _MAGIC + 24.0, Op.add, Op.min)
                sk = sb.tile([_FP, W], f32, tag="sk")
                nc.vector.scalar_tensor_tensor(sk[:], kc1[:], _MAGIC, SB[:],
                                               Op.subtract, Op.mult)
                val = sb.tile([_FP, W], f32, tag="val")
                nc.vector.tensor_tensor(val[:], U[:], sk[:], Op.subtract)
                vq = sb.tile([_FP, W], bf16, tag="vq")
                nc.scalar.activation(vq[:], val[:], Act.Square)

                m1 = sb.tile([_FP, W], bf16, tag="m1")
                nc.vector.tensor_tensor(m1[:], U[:], vs[:], Op.min)
                mm = sb.tile([_FP, _CHUNK], bf16, tag="mm")
                nc.vector.tensor_tensor(mm[:], m1[:, xh], m1[:, yh], Op.min)

                exy = sb.tile([_FP, W], bf16, tag="exy")
                nc.vector.tensor_tensor(exy[:], usq[:], vsq[:], Op.min)
                e1 = sb.tile([_FP, _CHUNK], bf16, tag="e1")
                nc.vector.tensor_tensor(e1[:], exy[:, xh], vq[:, yh], Op.add)
                e2 = sb.tile([_FP, _CHUNK], bf16, tag="e2")
                nc.vector.tensor_tensor(e2[:], exy[:, yh], vq[:, xh], Op.add)
                dmin = sb.tile([_FP, _CHUNK], bf16, tag="dmin")
                nc.vector.tensor_tensor(dmin[:], e1[:], e2[:], Op.min)

                scr = sb.tile([_FP, _CHUNK], f32, tag="scr")
                nc.vector.scalar_tensor_tensor(
                    scr[:], mm[:], 0.0, dmin[:], Op.is_lt, Op.mult,
                    accum_out=part[:, c:c + 1])

            nc.sync.dma_start(out_d[:], part[:])
    nc.compile()
    return nc


def _combo_cols(A, T):
    """Per-row combo vectors (f32) for fragment-source boxes A vs grid
    boxes T.  Returns dict of [N] arrays."""
    A = A.astype(np.float32, copy=False)
    T = T.astype(np.float32, copy=False)
    w = A[:, 2] - A[:, 0]
    h = A[:, 3] - A[:, 1]
    tw = T[:, 2] - T[:, 0]
    th = T[:, 3] - T[:, 1]
    dx = A[:, 0] - T[:, 0]
    dy = A[:, 1] - T[:, 1]
    dvx = T[:, 2] - A[:, 0]
    dvy = T[:, 3] - A[:, 1]
    with np.errstate(divide="ignore"):
        rix = np.where(tw != 0, np.float32(24.0) / tw, np.float32(0.0))
        riy = np.where(th != 0, np.float32(24.0) / th, np.float32(0.0))
    rix = rix.astype(np.float32)
    riy = riy.astype(np.float32)
    return dict(
        w=w, dx=dx, nw=-w, dvx=dvx, wrx=w * rix, dxrx=dx * rix,
        h=h, dy=dy, nh=-h, dvy=dvy, wry=h * riy, dyry=dy * riy,
        sx=tw / np.float32(24.0), sy=th / np.float32(24.0))


def _mmin_for_core(boxes_c, targets_c):
    """Build the [6, _MMW] matmul-input DRAM tensor for one core."""
    d1 = _combo_cols(boxes_c, targets_c)     # dir1: cols 0:512
    d2 = _combo_cols(targets_c, boxes_c)     # dir2: cols 512:1024
    cat = {k: np.concatenate([d1[k], d2[k]]) for k in d1}

    m = np.zeros((6, _MMW), dtype=np.float32)
    m[0, :_FP] = np.repeat(_LIN10, 10)       # gx  (i of f = i*10+j)
    m[1, :_FP] = 1.0
    m[2, :_FP] = np.tile(_LIN10, 10)         # gy
    m[3, :_FP] = 1.0
    m[4, :_FP] = 1.0                         # ones (K=1 lhsT)
    B0, B1, B2 = (slice(_LW + b * _COLS, _LW + (b + 1) * _COLS)
                  for b in range(3))
    m[0, B0] = cat["w"];    m[1, B0] = cat["dx"]
    m[0, B1] = cat["nw"];   m[1, B1] = cat["dvx"]
    m[0, B2] = cat["wrx"];  m[1, B2] = cat["dxrx"]
    m[2, B0] = cat["h"];    m[3, B0] = cat["dy"]
    m[2, B1] = cat["nh"];   m[3, B1] = cat["dvy"]
    m[2, B2] = cat["wry"];  m[3, B2] = cat["dyry"]
    m[4, _LW:_LW + _COLS] = cat["sx"]
    m[4, _LW + _COLS:_LW + 2 * _COLS] = cat["sy"]
    return m


def kernel(boxes: np.ndarray, targets: np.ndarray) -> np.ndarray:
    from concourse.bass_utils import run_bass_kernel_spmd

    global LAST_RESULTS
    boxes = np.ascontiguousarray(boxes, dtype=np.float32)
    targets = np.ascontiguousarray(targets, dtype=np.float32)
    assert boxes.shape == (_B, 4) and targets.shape == (_B, 4)

    if "nc" not in _compiled:
        _compiled["nc"] = _build_nc()
    nc = _compiled["nc"]

    in_maps = []
    for c in range(_N_CORES):
        rows = slice(c * _BOX_PER_CORE, (c + 1) * _BOX_PER_CORE)
        in_maps.append({"mmin": _mmin_for_core(boxes[rows], targets[rows])})

    trace = bool(int(os.environ.get("BOXLOSS_TRACE", "0")))
    res = run_bass_kernel_spmd(nc, in_maps, list(range(_N_CORES)),
                               trace=trace)
    LAST_RESULTS = res

    total = np.float64(0.0)
    for r in res.results:
        total += r["out"].astype(np.float64).sum()
    loss = total / (2.0 * _B * _FP)
    return np.array(loss, dtype=np.float32)



# revision 5
# speedup vs baseline: 1.0411x; 1.0411x over previous
"""BoxRenderLoss Trainium2 kernel (v3: rows-on-partitions, 10-pt coord grids).

loss = mean over (box, fragment) pairs of masked min-squared-distance between
each box's 10x10 fragment grid and the other box's 100-point sampled boundary,
both directions, / (2*B*FP).

Math: the min over the 100 boundary points decomposes into the 4 box edges;
each edge's 25-point uniform grid min has the closed form
k = clamp(round(u/s), 0, 24), val = u - s*k = -s*(k - T), T = u/s.  Key
structural win: the fragment grid is a 10x10 product grid, so every
per-coordinate quantity (u, v, u^2, v^2, val^2, min(u,v)) takes only 10
distinct values per axis per row; only the final combine runs over the full
100 fragments, via stride-0 broadcast access patterns.

The outside-mask multiply is folded into the min-chain:
  contribution = relu(min(e1, e2, -BIG*min(ux,vx,uy,vy)))
with BIG = 2^13: inside fragments give a nonpositive third term -> relu -> 0;
outside ones give a huge positive third term -> min picks dmin.  The final
relu runs on the Act engine with a free row-sum via accum_out.  BIG is a
power of two so the -BIG-scaled copies of u,v round identically in bf16 and
the mask decision is exactly consistent with the distance path.

Device layout: partitions = 128 virtual rows (4096 boxes x 2 directions,
data-parallel over 8 cores -> 1024 rows/core = 8 blocks of 128), free dim =
8 blocks x (10 x-grid | 10 y-grid) = 160 for coordinate ops, 8 x 10 x 10 =
800 for fragment-product ops.  U/V/Un come from bf16 K=4 outer-product
matmuls; T = u*(24/tw) from an f32r matmul (f32r: wrx,dxrx cancel).
k = clamp(round(T),0,24) via the 2^23 magic add (DVE) + Relu(x-2^23) (Act).
"""

import os
import numpy as np
import ml_dtypes

# Exact float32 bit patterns of jnp.linspace(0.0, 1.0, 10) (fragment grid).
_LIN10 = np.array(
    [0, 1038323257, 1046711865, 1051372203, 1055100473,
     1057896676, 1059760811, 1061624946, 1063489081, 1065353216],
    dtype=np.uint32,
).view(np.float32)

_B = 4096
_FP = 100
_N_CORES = 8
_BOX_PER_CORE = _B // _N_CORES          # 512
_ROWS = 2 * _BOX_PER_CORE               # 1024 virtual rows per core
_NBLK = _ROWS // 128                    # 8 blocks of 128 partitions
_CW = 20                                # coord cols per block (10 x | 10 y)
_CF = _NBLK * _CW                       # 160 coord cols total
_PF = _NBLK * 100                       # 800 product cols total
_MAGIC = 8388608.0                      # 2^23 round-to-nearest trick
_BIG = 8192.0                           # 2^13 mask scale

LAST_RESULTS = None  # BassKernelResults of the most recent run (for test.py)

_compiled = {}


def _build_nc():
    import concourse.bass as bass
    import concourse.bacc as bacc
    import concourse.tile as tile
    from concourse import mybir

    f32 = mybir.dt.float32
    f32r = mybir.dt.float32r
    bf16 = mybir.dt.bfloat16
    Op = mybir.AluOpType
    Act = mybir.ActivationFunctionType

    nc = bacc.Bacc("TRN2", target_bir_lowering=False, debug=False,
                   num_devices=_N_CORES)
    lbf_d = nc.dram_tensor("lbf", [4, _NBLK * 3 * 128], bf16,
                           kind="ExternalInput").ap()
    lfr_d = nc.dram_tensor("lfr", [4, _NBLK * 128], f32r,
                           kind="ExternalInput").ap()
    rbf_d = nc.dram_tensor("rbf", [4, _CW], bf16, kind="ExternalInput").ap()
    rfr_d = nc.dram_tensor("rfr", [4, _CW], f32r, kind="ExternalInput").ap()
    s2t_d = nc.dram_tensor("s2t", [128, _CF], bf16, kind="ExternalInput").ap()
    cb_d = nc.dram_tensor("cb", [128, 1], f32, kind="ExternalInput").ap()
    out_d = nc.dram_tensor("out", [128, 1], f32, kind="ExternalOutput").ap()

    with tile.TileContext(nc) as tc:
        with (
            tc.tile_pool(name="const", bufs=1) as const,
            tc.tile_pool(name="sb", bufs=1) as sb,
            tc.tile_pool(name="ps", bufs=1, space="PSUM") as ps,
        ):
            lbf = const.tile([4, _NBLK * 3 * 128], bf16)
            lfr = const.tile([4, _NBLK * 128], f32r)
            rbf = const.tile([4, _CW], bf16)
            rfr = const.tile([4, _CW], f32r)
            s2t = const.tile([128, _CF], bf16)
            cb = const.tile([128, 1], f32)
            nc.sync.dma_start(cb[:], cb_d[:])
            nc.sync.dma_start(lbf[:], lbf_d[:])
            nc.sync.dma_start(lfr[:], lfr_d[:])
            nc.sync.dma_start(rbf[:], rbf_d[:])
            nc.sync.dma_start(rfr[:], rfr_d[:])
            nc.sync.dma_start(s2t[:], s2t_d[:])
            part = const.tile([128, 1], f32)

            U = ps.tile([128, _CF], f32, tag="U")
            V = ps.tile([128, _CF], f32, tag="V")
            Un = ps.tile([128, _CF], f32, tag="Un")
            T = ps.tile([128, _CF], f32, tag="T")
            for b in range(_NBLK):
                cs = slice(b * _CW, (b + 1) * _CW)
                l0 = b * 384
                nc.tensor.matmul(U[:, cs], lbf[0:4, l0:l0 + 128], rbf[0:4, :])
                nc.tensor.matmul(V[:, cs], lbf[0:4, l0 + 128:l0 + 256],
                                 rbf[0:4, :])
                nc.tensor.matmul(Un[:, cs], lbf[0:4, l0 + 256:l0 + 384],
                                 rbf[0:4, :])
                nc.tensor.matmul(T[:, cs], lfr[0:4, b * 128:(b + 1) * 128],
                                 rfr[0:4, :])

            # --- coordinate-level ops, [128, 160] ---
            # k = clamp(round(T), 0, 24); d = k - T; val^2 = s^2 * d^2
            kc0 = sb.tile([128, _CF], f32, tag="kc0")
            nc.vector.tensor_scalar(kc0[:], T[:], _MAGIC, _MAGIC + 24.0,
                                    Op.add, Op.min)
            kc1 = sb.tile([128, _CF], bf16, tag="kc1")
            nc.scalar.activation(kc1[:], kc0[:], Act.Relu, bias=cb[:, 0:1])
            d = sb.tile([128, _CF], bf16, tag="d")
            nc.vector.tensor_tensor(d[:], kc1[:], T[:], Op.subtract)
            dsq = sb.tile([128, _CF], bf16, tag="dsq")
            nc.scalar.activation(dsq[:], d[:], Act.Square)
            vq = sb.tile([128, _CF], bf16, tag="vq")
            nc.vector.tensor_tensor(vq[:], dsq[:], s2t[:], Op.mult)

            usq = sb.tile([128, _CF], bf16, tag="usq")
            nc.scalar.activation(usq[:], U[:], Act.Square)
            vsq = sb.tile([128, _CF], bf16, tag="vsq")
            nc.scalar.activation(vsq[:], V[:], Act.Square)
            exy = sb.tile([128, _CF], bf16, tag="exy")
            nc.vector.tensor_tensor(exy[:], usq[:], vsq[:], Op.min)

            # mask path: m1n = max(-BIG*u, -BIG*v) = -BIG*min(u,v)
            vsn = sb.tile([128, _CF], bf16, tag="vsn")
            nc.scalar.activation(vsn[:], V[:], Act.Copy, scale=-_BIG)
            m1n = sb.tile([128, _CF], bf16, tag="m1n")
            nc.vector.tensor_tensor(m1n[:], Un[:], vsn[:], Op.max)

            # --- fragment-product ops, [128, 800] via broadcast views ---
            def cview(t, c, inner_j):
                # [128, 8, 10, 10] view of coord half c (0=x grid i, 1=y grid j)
                a = t[:].rearrange("p (b c t) -> p b c t", b=_NBLK, c=2)
                a = a[:, :, c, :]              # [128, 8, 10]
                if inner_j:                    # values indexed by j (inner)
                    a = a.unsqueeze(2)         # [128, 8, 1, 10]
                else:                          # values indexed by i (outer)
                    a = a.unsqueeze(3)         # [128, 8, 10, 1]
                return a.broadcast_to((128, _NBLK, 10, 10))

            def pview(t):
                return t[:].rearrange("p (b i j) -> p b i j", i=10, j=10)

            e1 = sb.tile([128, _PF], bf16, tag="e1")
            nc.vector.tensor_tensor(pview(e1), cview(vq, 1, True),
                                    cview(exy, 0, False), Op.add)
            e2 = sb.tile([128, _PF], bf16, tag="e2")
            nc.vector.tensor_tensor(pview(e2), cview(vq, 0, False),
                                    cview(exy, 1, True), Op.add)
            # mmx = -BIG * min over the 4 coord margins, x-part via Act-
            # materialized packed repeat so the max runs in 2x DVE mode.
            m1xr = sb.tile([128, _PF], bf16, tag="m1xr")
            nc.scalar.activation(pview(m1xr), cview(m1n, 0, False), Act.Copy)
            mmx = sb.tile([128, _PF], bf16, tag="mmx")
            nc.vector.tensor_tensor(pview(mmx), cview(m1n, 1, True),
                                    pview(m1xr), Op.max)
            dmin = sb.tile([128, _PF], bf16, tag="dmin")
            nc.vector.tensor_tensor(dmin[:], e1[:], e2[:], Op.min)
            tm = sb.tile([128, _PF], bf16, tag="tm")
            nc.vector.tensor_tensor(tm[:], dmin[:], mmx[:], Op.min)
            scr = sb.tile([128, _PF], bf16, tag="scr")
            nc.scalar.activation(scr[:], tm[:], Act.Relu,
                                 accum_out=part[:, 0:1])

            nc.sync.dma_start(out_d[:], part[:])
    nc.compile()
    return nc


def _core_inputs(boxes_c, targets_c):
    """Build the per-core DRAM input map (512 boxes -> 1024 virtual rows)."""
    A = np.concatenate([boxes_c, targets_c]).astype(np.float32)   # frag source
    T = np.concatenate([targets_c, boxes_c]).astype(np.float32)   # grid box
    w = A[:, 2] - A[:, 0]
    h = A[:, 3] - A[:, 1]
    tw = T[:, 2] - T[:, 0]
    th = T[:, 3] - T[:, 1]
    dx = A[:, 0] - T[:, 0]
    dy = A[:, 1] - T[:, 1]
    dvx = T[:, 2] - A[:, 0]
    dvy = T[:, 3] - A[:, 1]
    with np.errstate(divide="ignore"):
        rix = np.where(tw != 0, np.float32(24.0) / tw, np.float32(0.0))
        riy = np.where(th != 0, np.float32(24.0) / th, np.float32(0.0))
    rix = rix.astype(np.float32)
    riy = riy.astype(np.float32)
    nbig = np.float32(-_BIG)

    bf = ml_dtypes.bfloat16
    # lbf [4, 8*3*128]: per block, U-lhsT rows (w,dx,h,dy), V-lhsT rows
    # (-w,dvx,-h,dvy), Un-lhsT rows -BIG*(w,dx,h,dy).
    lbf = np.zeros((4, _NBLK * 3 * 128), dtype=np.float32)
    lfr = np.zeros((4, _NBLK * 128), dtype=np.float32)
    for b in range(_NBLK):
        rs = slice(b * 128, (b + 1) * 128)
        l0 = b * 384
        lbf[0, l0:l0 + 128] = w[rs]
        lbf[1, l0:l0 + 128] = dx[rs]
        lbf[2, l0:l0 + 128] = h[rs]
        lbf[3, l0:l0 + 128] = dy[rs]
        lbf[0, l0 + 128:l0 + 256] = -w[rs]
        lbf[1, l0 + 128:l0 + 256] = dvx[rs]
        lbf[2, l0 + 128:l0 + 256] = -h[rs]
        lbf[3, l0 + 128:l0 + 256] = dvy[rs]
        lbf[0, l0 + 256:l0 + 384] = nbig * w[rs]
        lbf[1, l0 + 256:l0 + 384] = nbig * dx[rs]
        lbf[2, l0 + 256:l0 + 384] = nbig * h[rs]
        lbf[3, l0 + 256:l0 + 384] = nbig * dy[rs]
        t0 = b * 128
        lfr[0, t0:t0 + 128] = w[rs] * rix[rs]
        lfr[1, t0:t0 + 128] = dx[rs] * rix[rs]
        lfr[2, t0:t0 + 128] = h[rs] * riy[rs]
        lfr[3, t0:t0 + 128] = dy[rs] * riy[rs]

    rhs = np.zeros((4, _CW), dtype=np.float32)
    rhs[0, 0:10] = _LIN10
    rhs[1, 0:10] = 1.0
    rhs[2, 10:20] = _LIN10
    rhs[3, 10:20] = 1.0

    sx = tw / np.float32(24.0)
    sy = th / np.float32(24.0)
    s2 = np.zeros((128, _NBLK, 2, 10), dtype=np.float32)
    for b in range(_NBLK):
        rs = slice(b * 128, (b + 1) * 128)
        s2[:, b, 0, :] = (sx[rs] * sx[rs])[:, None]
        s2[:, b, 1, :] = (sy[rs] * sy[rs])[:, None]

    return {
        "lbf": lbf.astype(bf),
        "lfr": lfr,
        "rbf": rhs.astype(bf),
        "rfr": rhs,
        "s2t": s2.reshape(128, _CF).astype(bf),
        "cb": np.full((128, 1), -_MAGIC, dtype=np.float32),
    }


def kernel(boxes: np.ndarray, targets: np.ndarray) -> np.ndarray:
    from concourse.bass_utils import run_bass_kernel_spmd

    global LAST_RESULTS
    boxes = np.ascontiguousarray(boxes, dtype=np.float32)
    targets = np.ascontiguousarray(targets, dtype=np.float32)
    assert boxes.shape == (_B, 4) and targets.shape == (_B, 4)

    if "nc" not in _compiled:
        _compiled["nc"] = _build_nc()
    nc = _compiled["nc"]

    in_maps = []
    for c in range(_N_CORES):
        rows = slice(c * _BOX_PER_CORE, (c + 1) * _BOX_PER_CORE)
        in_maps.append(_core_inputs(boxes[rows], targets[rows]))

    trace = bool(int(os.environ.get("BOXLOSS_TRACE", "0")))
    res = run_bass_kernel_spmd(nc, in_maps, list(range(_N_CORES)),
                               trace=trace)
    LAST_RESULTS = res

    total = np.float64(0.0)
    for r in res.results:
        total += r["out"].astype(np.float64).sum()
    loss = total / (2.0 * _B * _FP)
    return np.array(loss, dtype=np.float32)


# revision 7
# speedup vs baseline: 1.5476x; 1.4865x over previous
"""BoxRenderLoss Trainium2 kernel (v4: rows-on-partitions, 10-pt coord grids).

loss = mean over (box, fragment) pairs of masked min-squared-distance between
each box's 10x10 fragment grid and the other box's 100-point sampled boundary,
both directions, / (2*B*FP).

Math: the min over the 100 boundary points decomposes into the 4 box edges;
each edge's 25-point uniform grid min has the closed form
k = clamp(round(u/s), 0, 24), val = u - s*k = -s*(k - T), T = u/s.  Key
structural win: the fragment grid is a 10x10 product grid, so every
per-coordinate quantity (u, v, u^2, v^2, val^2, min(u,v)) takes only 10
distinct values per axis per row; only the final combine runs over the full
100 fragments, via stride-0 broadcast access patterns.

The outside-mask multiply is folded into the min-chain:
  contribution = relu(min(e1, e2, -BIG*min(ux,vx,uy,vy)))
with BIG = 2^13: inside fragments give a nonpositive third term -> relu -> 0;
outside ones a huge positive one -> min picks dmin.  The final relu+row-sum is
a 4x-mode DVE tensor_scalar with accum_out; a K=128 PE matmul against a ones
vector then collapses the 128 partials so the output DMA is one descriptor
(a [128,1] output costs ~9us of per-descriptor semaphore propagation).

All matmul weights are bf16 in ONE DMA; T = u*(24/tw) needs ~f32 accuracy
(wrx, dxrx cancel), so its lhsT carries hi/lo bf16 pairs (K=8 split matmul),
which beats f32r's 4-pass LdWeights+Matmult (213+213ns per block) ~5x.

Device layout: partitions = 128 virtual rows (4096 boxes x 2 directions,
data-parallel over 8 cores -> 1024 rows/core = 8 blocks of 128), free dim =
8 blocks x (10 x-grid | 10 y-grid) = 160 for coordinate ops, 8 x 10 x 10 =
800 for fragment-product ops.
"""

import os
import numpy as np
import ml_dtypes

# Exact float32 bit patterns of jnp.linspace(0.0, 1.0, 10) (fragment grid).
_LIN10 = np.array(
    [0, 1038323257, 1046711865, 1051372203, 1055100473,
     1057896676, 1059760811, 1061624946, 1063489081, 1065353216],
    dtype=np.uint32,
).view(np.float32)

_B = 4096
_FP = 100
_N_CORES = 8
_BOX_PER_CORE = _B // _N_CORES          # 512
_ROWS = 2 * _BOX_PER_CORE               # 1024 virtual rows per core
_NBLK = _ROWS // 128                    # 8 blocks of 128 partitions
_CW = 20                                # coord cols per block (10 x | 10 y)
_CF = _NBLK * _CW                       # 160 coord cols total
_PF = _NBLK * 100                       # 800 product cols total
_MAGIC = 8388608.0                      # 2^23 round-to-nearest trick
_BIG = 8192.0                           # 2^13 mask scale

LAST_RESULTS = None  # BassKernelResults of the most recent run (for test.py)

_compiled = {}


def _build_nc():
    import concourse.bass as bass
    import concourse.bacc as bacc
    import concourse.tile as tile
    from concourse import mybir

    f32 = mybir.dt.float32
    bf16 = mybir.dt.bfloat16
    Op = mybir.AluOpType
    Act = mybir.ActivationFunctionType

    nc = bacc.Bacc("TRN2", target_bir_lowering=False, debug=False,
                   num_devices=_N_CORES)
    # wts: per block b at col b*512: U lhsT (rows 0-3: w,dx,h,dy) | V lhsT
    # (rows 0-3) | Un lhsT (rows 0-3) | T lhsT (rows 0-7: hi/lo pairs).
    wts_d = nc.dram_tensor("wts", [8, _NBLK * 512], bf16,
                           kind="ExternalInput").ap()
    # rhs: cols 0-19 for U/V/Un (rows 0-3), cols 20-39 for T (rows 0-7).
    rhs_d = nc.dram_tensor("rhs", [8, 40], bf16, kind="ExternalInput").ap()
    s2t_d = nc.dram_tensor("s2t", [128, _CF], bf16, kind="ExternalInput").ap()
    out_d = nc.dram_tensor("out", [1, 1], f32, kind="ExternalOutput").ap()

    with tile.TileContext(nc) as tc:
        with (
            tc.tile_pool(name="const", bufs=1) as const,
            tc.tile_pool(name="sb", bufs=1) as sb,
            tc.tile_pool(name="ps", bufs=1, space="PSUM") as ps,
        ):
            wts = const.tile([8, _NBLK * 512], bf16)
            rhs = const.tile([8, 40], bf16)
            s2t = const.tile([128, _CF], bf16)
            nc.sync.dma_start(wts[:], wts_d[:])
            nc.sync.dma_start(rhs[:], rhs_d[:])
            nc.sync.dma_start(s2t[:], s2t_d[:])
            cb = const.tile([128, 1], f32)
            nc.gpsimd.memset(cb[:], -_MAGIC)
            ones = const.tile([128, 1], f32)
            nc.gpsimd.memset(ones[:], 1.0)
            part = const.tile([128, 1], f32)

            U = ps.tile([128, _CF], f32, tag="U")
            V = ps.tile([128, _CF], f32, tag="V")
            Un = ps.tile([128, _CF], f32, tag="Un")
            T = ps.tile([128, _CF], f32, tag="T")
            for b in range(_NBLK):
                cs = slice(b * _CW, (b + 1) * _CW)
                l0 = b * 512
                nc.tensor.matmul(T[:, cs], wts[0:8, l0 + 384:l0 + 512],
                                 rhs[0:8, 20:40])
            for b in range(_NBLK):
                cs = slice(b * _CW, (b + 1) * _CW)
                l0 = b * 512
                nc.tensor.matmul(U[:, cs], wts[0:4, l0:l0 + 128],
                                 rhs[0:4, 0:20])
                nc.tensor.matmul(V[:, cs], wts[0:4, l0 + 128:l0 + 256],
                                 rhs[0:4, 0:20])
                nc.tensor.matmul(Un[:, cs], wts[0:4, l0 + 256:l0 + 384],
                                 rhs[0:4, 0:20])

            # --- coordinate-level ops, [128, 160] ---
            # k = clamp(round(T), 0, 24); d = k - T; val^2 = s^2 * d^2
            kc0 = sb.tile([128, _CF], f32, tag="kc0")
            nc.vector.tensor_scalar(kc0[:], T[:], _MAGIC, _MAGIC + 24.0,
                                    Op.add, Op.min)
            kc1 = sb.tile([128, _CF], bf16, tag="kc1")
            nc.scalar.activation(kc1[:], kc0[:], Act.Relu, bias=cb[:, 0:1])
            d = sb.tile([128, _CF], bf16, tag="d")
            nc.vector.tensor_tensor(d[:], kc1[:], T[:], Op.subtract)
            dsq = sb.tile([128, _CF], bf16, tag="dsq")
            nc.scalar.activation(dsq[:], d[:], Act.Square)
            vq = sb.tile([128, _CF], bf16, tag="vq")
            nc.vector.tensor_tensor(vq[:], dsq[:], s2t[:], Op.mult)

            usq = sb.tile([128, _CF], bf16, tag="usq")
            nc.scalar.activation(usq[:], U[:], Act.Square)
            vsq = sb.tile([128, _CF], bf16, tag="vsq")
            nc.scalar.activation(vsq[:], V[:], Act.Square)
            exy = sb.tile([128, _CF], bf16, tag="exy")
            nc.vector.tensor_tensor(exy[:], usq[:], vsq[:], Op.min)

            # mask path: m1n = max(-BIG*u, -BIG*v) = -BIG*min(u,v)
            vsn = sb.tile([128, _CF], bf16, tag="vsn")
            nc.scalar.activation(vsn[:], V[:], Act.Copy, scale=-_BIG)
            m1n = sb.tile([128, _CF], bf16, tag="m1n")
            nc.vector.tensor_tensor(m1n[:], Un[:], vsn[:], Op.max)

            # --- fragment-product ops, [128, 800] via broadcast views ---
            def cview(t, c, inner_j):
                # [128, 8, 10, 10] view of coord half c (0=x grid i, 1=y grid j)
                a = t[:].rearrange("p (b c t) -> p b c t", b=_NBLK, c=2)
                a = a[:, :, c, :]              # [128, 8, 10]
                if inner_j:                    # values indexed by j (inner)
                    a = a.unsqueeze(2)         # [128, 8, 1, 10]
                else:                          # values indexed by i (outer)
                    a = a.unsqueeze(3)         # [128, 8, 10, 1]
                return a.broadcast_to((128, _NBLK, 10, 10))

            def pview(t):
                return t[:].rearrange("p (b i j) -> p b i j", i=10, j=10)

            e1 = sb.tile([128, _PF], bf16, tag="e1")
            nc.vector.tensor_tensor(pview(e1), cview(vq, 1, True),
                                    cview(exy, 0, False), Op.add)
            e2 = sb.tile([128, _PF], bf16, tag="e2")
            nc.vector.tensor_tensor(pview(e2), cview(vq, 0, False),
                                    cview(exy, 1, True), Op.add)
            # mmx = -BIG * min over the 4 coord margins; x-part via Act-
            # materialized packed repeat so the max runs in 2x DVE mode.
            m1xr = sb.tile([128, _PF], bf16, tag="m1xr")
            nc.scalar.activation(pview(m1xr), cview(m1n, 0, False), Act.Copy)
            mmx = sb.tile([128, _PF], bf16, tag="mmx")
            nc.vector.tensor_tensor(pview(mmx), cview(m1n, 1, True),
                                    pview(m1xr), Op.max)
            dmin = sb.tile([128, _PF], bf16, tag="dmin")
            nc.vector.tensor_tensor(dmin[:], e1[:], e2[:], Op.min)
            tm = sb.tile([128, _PF], bf16, tag="tm")
            nc.vector.tensor_tensor(tm[:], dmin[:], mmx[:], Op.min)
            scr = sb.tile([128, _PF], bf16, tag="scr")
            nc.vector.tensor_scalar(scr[:], tm[:], 0.0, None, Op.max,
                                    Op.add, accum_out=part[:, 0:1])

            # collapse 128 per-partition partials -> [1,1] so the output DMA
            # is a single descriptor
            sm = ps.tile([1, 1], f32, tag="sm")
            nc.tensor.matmul(sm[0:1, 0:1], part[:, 0:1], ones[:, 0:1])
            smc = sb.tile([1, 1], f32, tag="smc")
            nc.scalar.activation(smc[0:1, 0:1], sm[0:1, 0:1], Act.Copy)
            nc.sync.dma_start(out_d[:], smc[0:1, 0:1])
    nc.compile()
    return nc


def _bf16_hilo(v):
    bf = ml_dtypes.bfloat16
    hi = v.astype(bf)
    lo = (v - hi.astype(np.float32)).astype(bf)
    return hi, lo


def _core_inputs(boxes_c, targets_c):
    """Build the per-core DRAM input map (512 boxes -> 1024 virtual rows)."""
    A = np.concatenate([boxes_c, targets_c]).astype(np.float32)   # frag source
    T = np.concatenate([targets_c, boxes_c]).astype(np.float32)   # grid box
    w = A[:, 2] - A[:, 0]
    h = A[:, 3] - A[:, 1]
    tw = T[:, 2] - T[:, 0]
    th = T[:, 3] - T[:, 1]
    dx = A[:, 0] - T[:, 0]
    dy = A[:, 1] - T[:, 1]
    dvx = T[:, 2] - A[:, 0]
    dvy = T[:, 3] - A[:, 1]
    with np.errstate(divide="ignore"):
        rix = np.where(tw != 0, np.float32(24.0) / tw, np.float32(0.0))
        riy = np.where(th != 0, np.float32(24.0) / th, np.float32(0.0))
    rix = rix.astype(np.float32)
    riy = riy.astype(np.float32)
    nbig = np.float32(-_BIG)

    bf = ml_dtypes.bfloat16
    wts = np.zeros((8, _NBLK * 512), dtype=np.float32)
    for b in range(_NBLK):
        rs = slice(b * 128, (b + 1) * 128)
        l0 = b * 512
        wts[0, l0:l0 + 128] = w[rs]
        wts[1, l0:l0 + 128] = dx[rs]
        wts[2, l0:l0 + 128] = h[rs]
        wts[3, l0:l0 + 128] = dy[rs]
        wts[0, l0 + 128:l0 + 256] = -w[rs]
        wts[1, l0 + 128:l0 + 256] = dvx[rs]
        wts[2, l0 + 128:l0 + 256] = -h[rs]
        wts[3, l0 + 128:l0 + 256] = dvy[rs]
        wts[0, l0 + 256:l0 + 384] = nbig * w[rs]
        wts[1, l0 + 256:l0 + 384] = nbig * dx[rs]
        wts[2, l0 + 256:l0 + 384] = nbig * h[rs]
        wts[3, l0 + 256:l0 + 384] = nbig * dy[rs]
    wtsb = wts.astype(bf)
    for b in range(_NBLK):
        rs = slice(b * 128, (b + 1) * 128)
        l0 = b * 512
        for row, v in ((0, w * rix), (2, dx * rix),
                       (4, h * riy), (6, dy * riy)):
            hi, lo = _bf16_hilo(v[rs].astype(np.float32))
            wtsb[row, l0 + 384:l0 + 512] = hi
            wtsb[row + 1, l0 + 384:l0 + 512] = lo

    rhs = np.zeros((8, 40), dtype=np.float32)
    rhs[0, 0:10] = _LIN10
    rhs[1, 0:10] = 1.0
    rhs[2, 10:20] = _LIN10
    rhs[3, 10:20] = 1.0
    rhs[0, 20:30] = _LIN10
    rhs[1, 20:30] = _LIN10
    rhs[2, 20:30] = 1.0
    rhs[3, 20:30] = 1.0
    rhs[4, 30:40] = _LIN10
    rhs[5, 30:40] = _LIN10
    rhs[6, 30:40] = 1.0
    rhs[7, 30:40] = 1.0

    sx = tw / np.float32(24.0)
    sy = th / np.float32(24.0)
    s2 = np.zeros((128, _NBLK, 2, 10), dtype=np.float32)
    for b in range(_NBLK):
        rs = slice(b * 128, (b + 1) * 128)
        s2[:, b, 0, :] = (sx[rs] * sx[rs])[:, None]
        s2[:, b, 1, :] = (sy[rs] * sy[rs])[:, None]

    return {
        "wts": wtsb,
        "rhs": rhs.astype(bf),
        "s2t": s2.reshape(128, _CF).astype(bf),
    }


def kernel(boxes: np.ndarray, targets: np.ndarray) -> np.ndarray:
    from concourse.bass_utils import run_bass_kernel_spmd

    global LAST_RESULTS
    boxes = np.ascontiguousarray(boxes, dtype=np.float32)
    targets = np.ascontiguousarray(targets, dtype=np.float32)
    assert boxes.shape == (_B, 4) and targets.shape == (_B, 4)

    if "nc" not in _compiled:
        _compiled["nc"] = _build_nc()
    nc = _compiled["nc"]

    in_maps = []
    for c in range(_N_CORES):
        rows = slice(c * _BOX_PER_CORE, (c + 1) * _BOX_PER_CORE)
        in_maps.append(_core_inputs(boxes[rows], targets[rows]))

    trace = bool(int(os.environ.get("BOXLOSS_TRACE", "0")))
    res = run_bass_kernel_spmd(nc, in_maps, list(range(_N_CORES)),
                               trace=trace)
    LAST_RESULTS = res

    total = np.float64(0.0)
    for r in res.results:
        total += np.float64(r["out"][0, 0])
    loss = total / (2.0 * _B * _FP)
    return np.array(loss, dtype=np.float32)
